# revision 23
# baseline (speedup 1.0000x reference)
"""Trainium2 Bass kernel for nn_ConvAttention_34600256537137.

Math notes (validated against the reference):
  qkv = 1x1conv(x, w1)+b1 -> Q,K,V;  score = conv5x5(Q_s)+conv5x5(K_t)+b2;
  attn = softmax_t(score);  out = einsum(attn, V).
  Softmax over t is shift-invariant, so the Q-half of the score (constant in
  t), b2, and the K-path bias all cancel.  The computation collapses to:
    weff[ci,dy,dx] = sum_c w1K[c,ci] * w2K[c,dy,dx]        (host, tiny)
    sK[b,t,h,w]    = conv5x5_reflect(x[b,:,:,:,t], weff)
    e = exp(sK);  den = sum_t e
    out[b,o,h,w,s] = (sum_{ci,t} w1V[o,ci] * e * x) / den + b1V[o]
  (s-independent; normalization + S-broadcast + bias done on host)

Sharding: 8 cores = (b in {0,1}) x (4 chunks of 8 rows of H).

Perf structure (v7):
  - bf16 matmuls (PSUM accumulates fp32); 4x PE rate vs fp32.
  - slab arrives as 12 single-row DMAs so conv matmuls start early and are
    never starved by a big transfer.
  - phase 1 (score conv): T[tap, pos] = weff^T @ slab in 14 contiguous
    512-col matmuls; contiguous PSUM->SBUF cast copies alternate DVE/Act;
    slab free layout (row~, t, w~).
  - T bounced to DRAM in [tap, row~, t, w~] layout: the (row,t) partition
    pair merges to a single stride-36 dim, dy folds into the DMA offset and
    dx becomes a stride-(NPOS+1) dim -> the 25 shift-gathers collapse to
    5 DMAs (one per dy).  td writes + gathers issue on sync/gpsimd only
    (scalar is busy with conv copies) in readiness order.
  - tap reduce runs as 5 per-dy partial reduces + a small add tree, each
    pipelined behind its gather's completion.
  - sK (fp32, exact) is bounced to DRAM in (t,row,w) order; ONE strided
    read with a 0-stride ci8-replication dim rebroadcasts it to all 128
    partitions; exp runs after the bounce.  The denominator is then a
    selector-matmul on the rebroadcast e (rows g==0), which directly yields
    1/den in the output layout [64, 256] - no hsel/dend bounces, and the
    den/recip chain runs concurrently with the V path.
  - V path: xattn = x_t * e_b (DVE, 2 chunks); contract (ci,t) on PE in 8
    bf16 matmuls; normalization folded into the PSUM->SBUF read, split in
    partition halves so the fp16 output DMAs start early.
  - output is [C, HW] fp16 (S-broadcast + bias on host): 32KB not 1MB.
"""

import sys

if "/opt/trn_rl_repo" not in sys.path:
    sys.path.insert(0, "/opt/trn_rl_repo")

import numpy as np
import ml_dtypes

BF16 = ml_dtypes.bfloat16

B, C, H, W, S = 2, 64, 32, 32, 16
KS, PAD = 5, 2
NCORES = 8
ROWS = H // 4            # output rows per core
SLAB_R = ROWS + 2 * PAD  # 12
SLAB_W = W + 2 * PAD     # 36
NTAP = KS * KS           # 25
NPOS = SLAB_R * SLAB_W * S  # 6912 slab positions per tap
HW = ROWS * W            # 256 output positions
RT = SLAB_W * S          # 576 = one row~'s (t, w~) block
CH = 512                 # matmul chunk (free cols); 6912 = 13*512 + 256
NCH = 14

_MODULE = None


def _build_module():
    import concourse.bacc as bacc
    import concourse.bass as bass
    import concourse.tile as tile
    from concourse import mybir

    f32 = mybir.dt.float32
    f16 = mybir.dt.float16
    bf16 = mybir.dt.bfloat16
    AF = mybir.ActivationFunctionType
    ALU = mybir.AluOpType
    nc = bacc.Bacc("TRN2", target_bir_lowering=False, debug=False, num_devices=NCORES)

    # slab free layout per channel partition: (row~, t, w~) flat = 6912
    slab_d = nc.dram_tensor("slab", [C, SLAB_R, S, SLAB_W], bf16, kind="ExternalInput")
    xt_d = nc.dram_tensor("xt", [128, 8, HW], bf16, kind="ExternalInput")
    weff_d = nc.dram_tensor("weff", [C, NTAP], bf16, kind="ExternalInput")
    w1vr_d = nc.dram_tensor("w1vr", [128, 8, C], bf16, kind="ExternalInput")
    hsel_d = nc.dram_tensor("hsel", [128, ROWS], bf16, kind="ExternalInput")
    o_d = nc.dram_tensor("o", [C, HW], f16, kind="ExternalOutput")

    # scratch DRAM for partition-crossing rearrangements
    td_d = nc.dram_tensor("td", [NTAP, SLAB_R, S, SLAB_W], bf16)  # T, tap-major
    skd_d = nc.dram_tensor("skd", [S, ROWS, W], f16)              # sK, (t,row,w)
    dend_d = nc.dram_tensor("dend", [ROWS, W], bf16)              # 1/den

    with tile.TileContext(nc) as tc:
        with tc.tile_pool(name="sb", bufs=1) as sb, tc.tile_pool(
            name="ps", bufs=6, space="PSUM"
        ) as ps, tc.tile_pool(name="pso", bufs=1, space="PSUM") as pso:
            # --- loads: weff + slab rows in order; single-row DMAs ---
            s_slab = sb.tile([C, SLAB_R, S, SLAB_W], bf16)
            s_weff = sb.tile([C, NTAP], bf16)
            s_hsel = sb.tile([128, ROWS], bf16)
            s_xt = sb.tile([128, 8, HW], bf16)
            s_w1vr = sb.tile([128, 8, C], bf16)
            nc.scalar.dma_start(s_weff, weff_d.ap())
            for r in range(SLAB_R):
                eng = (nc.sync, nc.scalar, nc.gpsimd)[r % 3]
                eng.dma_start(s_slab[:, r : r + 1], slab_d.ap()[:, r : r + 1])
            nc.sync.dma_start(s_xt[:, 0:4, :], xt_d.ap()[:, 0:4, :])
            nc.scalar.dma_start(s_xt[:, 4:8, :], xt_d.ap()[:, 4:8, :])
            nc.gpsimd.dma_start(s_hsel, hsel_d.ap())
            nc.gpsimd.dma_start(s_w1vr, w1vr_d.ap())

            # --- phase 1: T[tap, (row~, t, w~)] = weff^T @ slab ---
            s_T2 = sb.tile([NTAP, SLAB_R, S, SLAB_W], bf16)
            slab_flat = s_slab[:].rearrange("c a b d -> c (a b d)")
            t2_flat = s_T2[:].rearrange("k a b d -> k (a b d)")
            copy_engs = (nc.vector, nc.scalar)
            for mi in range(NCH):
                f0 = mi * CH
                f1 = min(f0 + CH, NPOS)
                p_t = ps.tile([NTAP, CH], f32, tag="pt")
                nc.tensor.matmul(
                    p_t[:, 0 : f1 - f0],
                    s_weff,
                    slab_flat[:, f0:f1],
                    start=True,
                    stop=True,
                )
                eng = copy_engs[mi % 2]
                if eng is nc.scalar:
                    eng.copy(t2_flat[:, f0:f1], p_t[:, 0 : f1 - f0])
                else:
                    eng.tensor_copy(t2_flat[:, f0:f1], p_t[:, 0 : f1 - f0])

            # --- T to DRAM (6 row-pair writes) then 5 dy-gathers on
            # sync/gpsimd only, readiness order.
            # td addr of elem (tap=(5dy+dx), row+dy, t, dx+w)
            #   = dy*(5*NPOS + RT) + dx*(NPOS + 1) + 36*(16*row + t) + w
            s_R = sb.tile([128, NTAP, W], bf16)
            for ci in range(6):
                (nc.sync, nc.gpsimd)[ci % 2].dma_start(
                    td_d.ap()[:, 2 * ci : 2 * ci + 2], s_T2[:, 2 * ci : 2 * ci + 2]
                )
            for dy in range(KS):
                src = bass.AP(
                    tensor=td_d.ap().tensor,
                    offset=dy * (KS * NPOS + RT),
                    ap=[[SLAB_W, 128], [NPOS + 1, KS], [1, W]],
                )
                (nc.sync, nc.gpsimd)[0 if dy in (0, 2) else 1].dma_start(
                    s_R[:, dy * KS : (dy + 1) * KS], src
                )

            # --- tap reduce: per-dy partials pipelined behind gathers ---
            s_p = [
                sb.tile([128, W], f32, name=f"s_p{dy}") for dy in range(KS)
            ]
            s_a01 = sb.tile([128, W], f32)
            s_a23 = sb.tile([128, W], f32)
            s_a03 = sb.tile([128, W], f32)
            s_sk = sb.tile([128, W], f16)   # [(row,t), w]
            for dy in range(KS):
                nc.vector.tensor_reduce(
                    s_p[dy],
                    s_R[:, dy * KS : (dy + 1) * KS].transpose([0, 2, 1]),
                    axis=mybir.AxisListType.X,
                    op=ALU.add,
                )
                if dy == 1:
                    nc.vector.tensor_tensor(s_a01, s_p[0], s_p[1], op=ALU.add)
                if dy == 3:
                    nc.vector.tensor_tensor(s_a23, s_p[2], s_p[3], op=ALU.add)
                    nc.vector.tensor_tensor(s_a03, s_a01, s_a23, op=ALU.add)
            with nc.allow_low_precision(reason="fp16 sK bounce; tol is 2e-2"):
                nc.vector.tensor_tensor(s_sk, s_a03, s_p[4], op=ALU.add)

            # --- bounce sK to DRAM (t,row,w); one 0-stride-replicated read
            # rebroadcasts to all 128 partitions; exp after the bounce ---
            nc.sync.dma_start(
                bass.AP(
                    tensor=skd_d.ap().tensor,
                    offset=0,
                    ap=[[W, ROWS], [ROWS * W, S], [1, W]],
                ),
                s_sk,
            )
            s_skb = sb.tile([128, HW], f16)
            nc.sync.dma_start(
                s_skb[0:64],
                bass.AP(
                    tensor=skd_d.ap().tensor,
                    offset=0,
                    ap=[[0, 4], [ROWS * W, S], [1, HW]],
                ),
            )
            nc.gpsimd.dma_start(
                s_skb[64:128],
                bass.AP(
                    tensor=skd_d.ap().tensor,
                    offset=0,
                    ap=[[0, 4], [ROWS * W, S], [1, HW]],
                ),
            )

            # --- den path, fully off the critical chain: small exp on the
            # pre-bounce sK, indicator-matmul, recip [8,32], tiny bounce ---
            s_e32 = sb.tile([128, W], bf16)
            nc.scalar.activation(s_e32, s_sk, AF.Exp)
            p_den = pso.tile([ROWS, W], f32, tag="den")
            nc.tensor.matmul(p_den, s_hsel, s_e32, start=True, stop=True)
            s_rcp = sb.tile([ROWS, W], bf16)
            with nc.allow_low_precision(reason="1/den fits bf16; tol is 2e-2"):
                nc.vector.reciprocal(s_rcp, p_den)
            nc.sync.dma_start(dend_d.ap(), s_rcp)
            s_rcpb = sb.tile([C, HW], bf16)
            nc.sync.dma_start(
                s_rcpb,
                bass.AP(tensor=dend_d.ap().tensor, offset=0, ap=[[0, C], [1, HW]]),
            )

            # exp on the rebroadcast sK (the critical chain)
            s_eb = sb.tile([128, HW], bf16)
            nc.scalar.activation(s_eb, s_skb, AF.Exp)

            # --- V path: xattn = x_t * e; contract (ci,t) on PE ---
            s_xa = sb.tile([128, 8, HW], bf16)
            p_o = pso.tile([C, HW], f32, tag="out")
            for half in range(2):
                g0, g1 = 4 * half, 4 * half + 4
                nc.vector.tensor_tensor(
                    s_xa[:, g0:g1, :],
                    s_xt[:, g0:g1, :],
                    s_eb.unsqueeze(1).broadcast_to((128, 4, HW)),
                    op=ALU.mult,
                )
                for g in range(g0, g1):
                    nc.tensor.matmul(
                        p_o,
                        s_w1vr[:, g, :],
                        s_xa[:, g, :],
                        start=(g == 0),
                        stop=(g == 7),
                    )
            # normalize on the PSUM->SBUF read; split so out DMAs start early
            s_o = sb.tile([C, HW], f16)
            with nc.allow_low_precision(reason="fp16 out; tol is 2e-2"):
                nc.vector.tensor_tensor(
                    s_o[0:32], p_o[0:32], s_rcpb[0:32], op=ALU.mult
                )
                nc.sync.dma_start(o_d.ap()[0:32], s_o[0:32])
                nc.vector.tensor_tensor(
                    s_o[32:64], p_o[32:64], s_rcpb[32:64], op=ALU.mult
                )
                nc.scalar.dma_start(o_d.ap()[32:64], s_o[32:64])

    nc.compile()
    return nc


def _get_module():
    global _MODULE
    if _MODULE is None:
        _MODULE = _build_module()
    return _MODULE


def make_host_inputs(x, w1, b1, w2, b2):
    """Host-side precompute: folded weights + per-core reflect-padded slices."""
    x = np.ascontiguousarray(np.asarray(x, np.float32))
    w1 = np.asarray(w1, np.float32)
    w2 = np.asarray(w2, np.float32)

    w1K = w1[C : 2 * C, :, 0, 0]          # [c, ci]
    w2K = w2[0, C : 2 * C]                # [c, 5, 5]
    weff = np.ascontiguousarray(
        np.einsum("ci,cyx->iyx", w1K, w2K).reshape(C, NTAP)
    ).astype(BF16)
    w1V = w1[2 * C :, :, 0, 0]            # [co, ci]

    # w1vr[(ci8,t), g, co] = w1V[co, 8g+ci8]
    tmp = w1V.T.reshape(8, 8, C)                      # (g, ci8, co)
    w1vr = np.ascontiguousarray(
        np.broadcast_to(tmp[:, :, None, :], (8, 8, S, C))
        .transpose(1, 2, 0, 3)
        .reshape(128, 8, C)
    ).astype(BF16)

    # hsel[(row,t), m] = 1 if row == m  (partition index = row*S + t)
    hsel = np.zeros((128, ROWS), np.float32)
    for r in range(ROWS):
        hsel[r * S : (r + 1) * S, r] = 1.0
    hsel = hsel.astype(BF16)

    in_maps = []
    for core in range(NCORES):
        b, hc = divmod(core, 4)
        h0 = ROWS * hc
        xp = np.pad(x[b], ((0, 0), (PAD, PAD), (PAD, PAD), (0, 0)), mode="reflect")
        # slab[c, row~, t, w~]
        slab = np.ascontiguousarray(
            xp[:, h0 : h0 + SLAB_R, :, :].transpose(0, 1, 3, 2)
        ).astype(BF16)
        xs = x[b][:, h0 : h0 + ROWS, :, :]            # [ci, h, w, t]
        xt = np.ascontiguousarray(
            xs.reshape(8, 8, ROWS, W, S)
            .transpose(1, 4, 0, 2, 3)
            .reshape(128, 8, HW)
        ).astype(BF16)
        in_maps.append(
            {"slab": slab, "xt": xt, "weff": weff, "w1vr": w1vr, "hsel": hsel}
        )
    return in_maps


def assemble_output(results, b1):
    b1V = np.asarray(b1, np.float32)[2 * C :]
    out = np.empty((B, C, H, W, S), np.float32)
    for core in range(NCORES):
        b, hc = divmod(core, 4)
        h0 = ROWS * hc
        o = results[core]["o"].astype(np.float32).reshape(C, ROWS, W, 1)
        out[b, :, h0 : h0 + ROWS, :, :] = o
    out += b1V[None, :, None, None, None]
    return out


def kernel(x, w1, b1, w2, b2):
    from concourse.bass_utils import run_bass_kernel_spmd

    nc = _get_module()
    in_maps = make_host_inputs(x, w1, b1, w2, b2)
    res = run_bass_kernel_spmd(nc, in_maps, core_ids=list(range(NCORES)))
    return assemble_output(res.results, b1)
